# revision 1
# baseline (speedup 1.0000x reference)
"""ChebConv(K=3) x2 + BN GNN kernel for 8 Trainium2 NeuronCores.

Strategy:
  - Nodes dst-sharded across 8 cores (12500 each, padded to 12544 = 98*128).
  - ChebNet algebra refactored: out = x@(W0-W2) + L(x@W1) + L(L(x@(2W2)))
    with L = -D^-1/2 A D^-1/2 applied as: scale rows by dis on the way into
    the gather table, scale by -dis on the way out of the aggregation.
  - Each propagate: per dst-tile (128 dsts), gather source rows from a
    replicated table in HBM via dma_gather (int16 idx => 4 row-chunks of
    <=32768), build a 0/1 selection matrix on DVE (iota==dstloc), and
    aggregate with TensorE matmuls accumulating in PSUM.
  - Tables replicated across cores via AllGather after each half-step.
  - Layer-1 tables fp16 (256B rows), layer-2 tables fp32 (64 feats = 256B).
"""
import sys
import time

for _p in ("/opt/trn_rl_repo",):
    if _p not in sys.path:
        sys.path.insert(0, _p)

import numpy as np

import concourse.bass as bass
import concourse.bacc as bacc
import concourse.mybir as mybir
import concourse.tile as tile
from concourse.masks import make_identity

N_CORES = 8
EPS = 1e-5
CHUNK = 32768  # dma_gather int16 index limit per chunk
NI_MAX_BATCHES = 8  # <=1024 idxs per dma_gather (HW ring limit)
SCRATCH = 16384  # SWDGE descriptor ring bytes/partition
WIDE_S = True  # one DVE selection-matrix build per gather call


def make_cfg(N, DIN, HID, OUT):
    SH = N // N_CORES
    assert SH * N_CORES == N
    TILES = (SH + 127) // 128
    SHP = TILES * 128
    TR = N_CORES * SHP  # table rows
    NCH = (TR + CHUNK - 1) // CHUNK
    return dict(N=N, DIN=DIN, HID=HID, OUT=OUT, SH=SH, SHP=SHP, TILES=TILES,
                TR=TR, NCH=NCH)


CFG = make_cfg(100000, 128, 128, 64)

# ---------------------------------------------------------------------------
# Host preprocessing
# ---------------------------------------------------------------------------


def preprocess_edges(edge_index, cfg):
    """Group edges by (dst shard, dst tile, src chunk). Stream order: for each
    tile-group (GSZ tiles), for each chunk, the group's tiles' padded slot
    runs contiguously -> gather calls of up to NI_MAX_BATCHES*128 idxs that
    span tiles. Builds a shared gather plan + per-core idx/dstloc streams."""
    N, SH, SHP, TILES, NCH = cfg["N"], cfg["SH"], cfg["SHP"], cfg["TILES"], cfg["NCH"]
    GSZ = 4
    src = edge_index[0].astype(np.int64)
    dst = edge_index[1].astype(np.int64)

    deg = np.bincount(src, minlength=N).astype(np.float64)
    dis = np.where(deg > 0, 1.0 / np.sqrt(np.maximum(deg, 1.0)), 0.0).astype(np.float32)

    shard = dst // SH
    tloc = (dst % SH) // 128
    rloc = (dst % SH) % 128
    rowof_all = (np.arange(N) // SH) * SHP + (np.arange(N) % SH)
    rowof = rowof_all[src]
    chunk = rowof // CHUNK

    key = (shard * TILES + tloc) * NCH + chunk
    order = np.argsort(key, kind="stable")
    grp_cnt = np.bincount(key, minlength=N_CORES * TILES * NCH)
    grp_start = np.zeros(N_CORES * TILES * NCH + 1, np.int64)
    np.cumsum(grp_cnt, out=grp_start[1:])
    nb = -(-grp_cnt.reshape(N_CORES, TILES, NCH) // 128)  # ceil
    nb_shared = nb.max(axis=0)  # [TILES, NCH]
    nb_shared[:, 0] = np.maximum(nb_shared[:, 0], 1)  # every tile >=1 batch

    B_total = int(nb_shared.sum())
    S_total = B_total * 128

    # ---- stream layout + gather plan ----
    # batches[gb] = (tile, chunk); slot offset of (t,ch) run start
    run_off = np.zeros((TILES, NCH), np.int64)
    batch_tile = np.zeros(B_total, np.int32)
    plan = []  # list of (chunk, slot0, nb_i, gb0)
    last_gb = np.zeros(TILES, np.int64)
    first_gb = np.full(TILES, -1, np.int64)
    pos = 0  # slot position
    gb = 0
    for g0 in range(0, TILES, GSZ):
        g1 = min(g0 + GSZ, TILES)
        for ch in range(NCH):
            run_nb = int(nb_shared[g0:g1, ch].sum())
            if run_nb == 0:
                continue
            # record run offsets per tile
            p = pos
            bstart = gb
            for t in range(g0, g1):
                run_off[t, ch] = p
                nbt = int(nb_shared[t, ch])
                for _ in range(nbt):
                    batch_tile[gb] = t
                    if first_gb[t] < 0:
                        first_gb[t] = gb
                    last_gb[t] = gb
                    gb += 1
                p += nbt * 128
            # gather calls covering this run
            sub = 0
            while sub < run_nb:
                nb_i = min(NI_MAX_BATCHES, run_nb - sub)
                plan.append((ch, pos + sub * 128, nb_i, bstart + sub))
                sub += nb_i
            pos += run_nb * 128
    assert pos == S_total and gb == B_total

    idx_stream = np.zeros((N_CORES, S_total), np.int16)
    dloc_stream = np.full((N_CORES, S_total), 255, np.int16)
    src_local = (rowof % CHUNK).astype(np.int16)
    s_sorted = src_local[order]
    r_sorted = rloc[order].astype(np.int16)

    for c in range(N_CORES):
        for t in range(TILES):
            for ch in range(NCH):
                g = (c * TILES + t) * NCH + ch
                n = grp_cnt[g]
                if n == 0:
                    continue
                a = grp_start[g]
                o = run_off[t, ch]
                idx_stream[c, o:o + n] = s_sorted[a:a + n]
                dloc_stream[c, o:o + n] = r_sorted[a:a + n]

    idx_w = idx_stream.reshape(N_CORES, S_total // 16, 16).transpose(0, 2, 1)
    idx_w = np.ascontiguousarray(np.tile(idx_w, (1, 8, 1)))
    dloc_t = dloc_stream.reshape(N_CORES, B_total, 128).transpose(0, 2, 1)
    dloc_t = np.ascontiguousarray(dloc_t).astype(np.float16)

    meta = dict(nb_shared=nb_shared, B_total=B_total, S_total=S_total,
                plan=tuple(plan), batch_tile=tuple(batch_tile.tolist()),
                first_gb=tuple(first_gb.tolist()), last_gb=tuple(last_gb.tolist()))
    return meta, dis, idx_w, dloc_t


def build_host_inputs(x, dis, weights, cfg):
    """Per-core input tensors (excluding idx/dloc)."""
    (W1, b1, W2, b2, g1, beta1, m1, v1, g2, beta2, m2, v2) = weights
    N, SH, SHP, TILES, TR = cfg["N"], cfg["SH"], cfg["SHP"], cfg["TILES"], cfg["TR"]
    DIN, HID, OUT = cfg["DIN"], cfg["HID"], cfg["OUT"]

    Wcat1 = np.concatenate([W1[0] - W1[2], W1[1], 2.0 * W1[2]], axis=1).astype(np.float16)
    Wcat2 = np.concatenate([W2[0] - W2[2], W2[1], 2.0 * W2[2]], axis=1).astype(np.float16)
    A1 = (g1 / np.sqrt(v1 + EPS)).astype(np.float32)
    C1 = (beta1 + (b1 - m1) * A1).astype(np.float32)
    A2 = (g2 / np.sqrt(v2 + EPS)).astype(np.float32)
    C2 = (beta2 + (b2 - m2) * A2).astype(np.float32)
    AC1 = np.concatenate([np.tile(A1, (128, 1)), np.tile(C1, (128, 1))], axis=1)
    AC2 = np.concatenate([np.tile(A2, (128, 1)), np.tile(C2, (128, 1))], axis=1)

    tab0 = np.zeros((TR, DIN), np.float16)
    xp = np.zeros((N_CORES, SHP, DIN), np.float32)
    disp = np.zeros((N_CORES, SHP), np.float32)
    for c in range(N_CORES):
        xs = x[c * SH:(c + 1) * SH]
        xp[c, :SH] = xs
        disp[c, :SH] = dis[c * SH:(c + 1) * SH]
        tab0[c * SHP:c * SHP + SH] = (dis[c * SH:(c + 1) * SH, None] * xs).astype(np.float16)

    in_maps = []
    for c in range(N_CORES):
        d = disp[c].reshape(TILES, 128).T  # [128, TILES]
        in_maps.append({
            "xT": np.ascontiguousarray(xp[c].T).astype(np.float16),
            "tab0": tab0,
            "disP": np.ascontiguousarray(d),
            "disN": np.ascontiguousarray(-d),
            "dis2N": np.ascontiguousarray(-(d.astype(np.float64) ** 2)).astype(np.float32),
            "Wcat1": Wcat1,
            "Wcat2": Wcat2,
            "AC1": AC1.astype(np.float32),
            "AC2": AC2.astype(np.float32),
        })
    return in_maps


# ---------------------------------------------------------------------------
# Bass program
# ---------------------------------------------------------------------------


def build_program(cfg, meta, repeat=1, parts="all", single=False, do_compile=True):
    dt = mybir.dt
    f16, f32, i16 = dt.float16, dt.float32, dt.int16
    SHP, TILES, TR, NCH = cfg["SHP"], cfg["TILES"], cfg["TR"], cfg["NCH"]
    DIN, HID, OUT = cfg["DIN"], cfg["HID"], cfg["OUT"]
    nb_shared = meta["nb_shared"]
    B_total, S_total = meta["B_total"], meta["S_total"]

    nc = bacc.Bacc("TRN2", target_bir_lowering=False, debug=False,
                   num_devices=(1 if single else N_CORES), num_swdge_queues=4,
                   dynamic_dma_scratch_size=SCRATCH)

    xT_d = nc.dram_tensor("xT", [128, SHP], f16, kind="ExternalInput")
    tab0_d = nc.dram_tensor("tab0", [TR, DIN], f16, kind="ExternalInput")
    disP_d = nc.dram_tensor("disP", [128, TILES], f32, kind="ExternalInput")
    disN_d = nc.dram_tensor("disN", [128, TILES], f32, kind="ExternalInput")
    dis2N_d = nc.dram_tensor("dis2N", [128, TILES], f32, kind="ExternalInput")
    W1_d = nc.dram_tensor("Wcat1", [DIN, 3 * HID], f16, kind="ExternalInput")
    W2_d = nc.dram_tensor("Wcat2", [HID, 3 * OUT], f16, kind="ExternalInput")
    AC1_d = nc.dram_tensor("AC1", [128, 2 * HID], f32, kind="ExternalInput")
    AC2_d = nc.dram_tensor("AC2", [128, 2 * OUT], f32, kind="ExternalInput")
    idx_d = nc.dram_tensor("idxs", [128, S_total // 16], i16, kind="ExternalInput")
    dloc_d = nc.dram_tensor("dloc", [128, B_total], f16, kind="ExternalInput")
    out_d = nc.dram_tensor("out", [SHP, OUT], f32, kind="ExternalOutput")

    rg = [list(range(N_CORES))]

    with tile.TileContext(nc) as tc:
        import contextlib
        ctx = contextlib.ExitStack()
        with ctx:
            const_p = ctx.enter_context(tc.tile_pool(name="const", bufs=1))
            big_p = ctx.enter_context(tc.tile_pool(name="big", bufs=1))
            g16_p = ctx.enter_context(tc.tile_pool(name="g16", bufs=20))
            s_p = ctx.enter_context(tc.tile_pool(name="sel", bufs=8))
            ev_p = ctx.enter_context(tc.tile_pool(name="ev", bufs=3))
            ps_prop = ctx.enter_context(tc.tile_pool(name="psprop", bufs=6, space="PSUM"))
            ps_dense = ctx.enter_context(tc.tile_pool(name="psdense", bufs=1, space="PSUM"))
            ps_tr = ctx.enter_context(tc.tile_pool(name="pstr", bufs=1, space="PSUM"))
            dram_p = ctx.enter_context(tc.tile_pool(name="dram", bufs=1, space="DRAM"))

            # ---- constants ----
            iota16_sb = const_p.tile([128, 128], f16)
            nc.gpsimd.iota(iota16_sb[:], pattern=[[1, 128]], base=0,
                           channel_multiplier=0,
                           allow_small_or_imprecise_dtypes=True)
            ident = const_p.tile([128, 128], f16)
            make_identity(nc, ident[:])
            W1_sb = const_p.tile([DIN, 3 * HID], f16)
            nc.sync.dma_start(out=W1_sb[:], in_=W1_d.ap())
            W2_sb = const_p.tile([HID, 3 * OUT], f16)
            nc.sync.dma_start(out=W2_sb[:], in_=W2_d.ap())
            AC1_sb = const_p.tile([128, 2 * HID], f32)
            nc.sync.dma_start(out=AC1_sb[:], in_=AC1_d.ap())
            AC2_sb = const_p.tile([128, 2 * OUT], f32)
            nc.sync.dma_start(out=AC2_sb[:], in_=AC2_d.ap())
            disP_sb = const_p.tile([128, TILES], f32)
            nc.sync.dma_start(out=disP_sb[:], in_=disP_d.ap())
            disN_sb = const_p.tile([128, TILES], f32)
            nc.sync.dma_start(out=disN_sb[:], in_=disN_d.ap())
            dis2N_sb = const_p.tile([128, TILES], f32)
            nc.sync.dma_start(out=dis2N_sb[:], in_=dis2N_d.ap())
            dloc16_sb = const_p.tile([128, B_total], f16)
            nc.sync.dma_start(out=dloc16_sb[:], in_=dloc_d.ap())
            idx_sb = const_p.tile([128, S_total // 16], i16)
            nc.sync.dma_start(out=idx_sb[:], in_=idx_d.ap())

            # ---- big resident arrays ----
            xT_sb = big_p.tile([128, SHP], f16, tag="xT")
            nc.sync.dma_start(out=xT_sb[:], in_=xT_d.ap())

            gq = [0]  # rotating gather queue
            rep_i = [0]

            def run_body():
                ri = rep_i[0]
                rep_i[0] += 1
                # ---- DRAM bounce + tables (fresh per repeat; Shared AG outs) ----
                za_sb = big_p.tile([128, TILES * HID], f16, tag="za")
                zb_sb = big_p.tile([128, TILES * HID], f16, tag="zb")
                b1_t = dram_p.tile([SHP, HID], f16, tag=f"b1_{ri}")
                t1_t = dram_p.tile([TR, HID], f16, addr_space="Shared", tag=f"t1_{ri}")
                b2_t = dram_p.tile([SHP, HID], f16, tag=f"b2_{ri}")
                t2_t = dram_p.tile([TR, HID], f16, addr_space="Shared", tag=f"t2_{ri}")
                b3_t = dram_p.tile([SHP, HID], f16, tag=f"b3_{ri}")
                t3_t = dram_p.tile([TR, HID], f16, addr_space="Shared", tag=f"t3_{ri}")
                b4_t = dram_p.tile([SHP, HID], f16, tag=f"b4_{ri}")
                t4_t = dram_p.tile([TR, HID], f16, addr_space="Shared", tag=f"t4_{ri}")

                def dense(lhs_sb, W_sb, F, za_dst, zb_dst, bounce, ev_dtype):
                    """z = lhs.T @ [Wa|Wb|Wc]; za kept, zb=dis*z_b kept, z_c=dis*z_c -> bounce."""
                    for t in range(TILES):
                        lhsT = lhs_sb[:, t * 128:(t + 1) * 128]
                        ps = ps_dense.tile([128, 3 * F], f32)
                        for j in range(3):
                            nc.tensor.matmul(ps[:, j * F:(j + 1) * F], lhsT,
                                             W_sb[:, j * F:(j + 1) * F],
                                             start=True, stop=True)
                        nc.vector.tensor_copy(za_dst[:, t * F:(t + 1) * F], ps[:, 0:F])
                        nc.vector.tensor_scalar(zb_dst[:, t * F:(t + 1) * F],
                                                ps[:, F:2 * F], disP_sb[:, t:t + 1],
                                                None, mybir.AluOpType.mult)
                        zc = ev_p.tile([128, 3 * OUT if F == OUT else F], ev_dtype, tag="zc")
                        nc.vector.tensor_scalar(zc[:, :F], ps[:, 2 * F:3 * F],
                                                disP_sb[:, t:t + 1], None,
                                                mybir.AluOpType.mult)
                        nc.sync.dma_start(out=bounce[t * 128:(t + 1) * 128, :],
                                          in_=zc[:, :F])

                em_g = parts in ("all", "gather", "gs")
                em_s = parts in ("all", "gs", "nogather")
                em_m = parts in ("all", "nogather")
                plan = meta["plan"]
                batch_tile = meta["batch_tile"]
                first_gb = meta["first_gb"]
                last_gb = meta["last_gb"]

                def propagate(table, F, gdt, g_pool, evac):
                    """y[dst] = sum_e table[src_e]; evac(t, psum) consumes PSUM."""
                    psums = {}
                    for (ch, slot0, nb_i, gb0) in plan:
                        rows0 = ch * CHUNK
                        rows1 = min((ch + 1) * CHUNK, TR)
                        ni = nb_i * 128
                        col0 = slot0 // 16
                        g = g_pool.tile([128, NI_MAX_BATCHES, F], gdt, tag="g")
                        if em_g:
                            nc.gpsimd.dma_gather(
                                out_ap=g[:, :nb_i, :], in_ap=table[rows0:rows1, :],
                                idxs_ap=idx_sb[:, col0:col0 + ni // 16], num_idxs=ni,
                                num_idxs_reg=ni, elem_size=F,
                                queue_num=gq[0] % 4)
                            gq[0] += 1
                        if em_s and WIDE_S:
                            Sw = s_p.tile([128, NI_MAX_BATCHES, 128], f16, tag="S")
                            nc.vector.tensor_tensor(
                                out=Sw[:, :nb_i, :],
                                in0=iota16_sb[:].unsqueeze(1).broadcast_to([128, nb_i, 128]),
                                in1=dloc16_sb[:, gb0:gb0 + nb_i].unsqueeze(2).broadcast_to([128, nb_i, 128]),
                                op=mybir.AluOpType.is_equal)
                        for b in range(nb_i):
                            gb = gb0 + b
                            t = batch_tile[gb]
                            if em_m:
                                lhs = Sw[:, b, :]
                                if t not in psums:
                                    psums[t] = ps_prop.tile([128, F], f32, tag="pp", name=f"pp_{t}")
                                nc.tensor.matmul(psums[t][:], lhs, g[:, b, :],
                                                 start=(gb == first_gb[t]),
                                                 stop=(gb == last_gb[t]))
                                if gb == last_gb[t]:
                                    evac(t, psums.pop(t))

                if not em_m:
                    # timing-only: 4 propagates' gather/S traffic vs input table
                    for _ in range(4):
                        propagate(tab0_d.ap(), HID, f16, g16_p, None)
                    return

                # ================= layer 1 =================
                dense(xT_sb, W1_sb, HID, za_sb, zb_sb, b1_t, np.float16 and f16)

                if single:
                    nc.sync.dma_start(out=t1_t[0:SHP, :], in_=b1_t[:, :])
                else:
                    nc.gpsimd.collective_compute(
                        "AllGather", mybir.AluOpType.bypass,
                        ins=[b1_t[:, :]], outs=[t1_t[:, :]], replica_groups=rg)

                def evac_p1(t, ps):
                    tmp = ev_p.tile([128, HID], f16, tag="tmp16")
                    nc.vector.tensor_scalar(tmp[:], ps[:], dis2N_sb[:, t:t + 1], None,
                                            mybir.AluOpType.mult)
                    v = ev_p.tile([128, HID], f16, tag="v16")
                    nc.vector.tensor_tensor(out=v[:], in0=tmp[:],
                                            in1=zb_sb[:, t * HID:(t + 1) * HID],
                                            op=mybir.AluOpType.add)
                    nc.sync.dma_start(out=b2_t[t * 128:(t + 1) * 128, :], in_=v[:])

                propagate(t1_t, HID, f16, g16_p, evac_p1)

                if single:
                    nc.sync.dma_start(out=t2_t[0:SHP, :], in_=b2_t[:, :])
                else:
                    nc.gpsimd.collective_compute(
                        "AllGather", mybir.AluOpType.bypass,
                        ins=[b2_t[:, :]], outs=[t2_t[:, :]], replica_groups=rg)

                hT_sb = big_p.tile([128, SHP], f16, tag=("xT" if repeat == 1 else "hT"))  # reuse xT slot

                def evac_p2(t, ps):
                    s1 = ev_p.tile([128, HID], f32, tag="s1")
                    nc.vector.tensor_scalar(s1[:], ps[:], disN_sb[:, t:t + 1], None,
                                            mybir.AluOpType.mult)
                    s2 = ev_p.tile([128, HID], f32, tag="s2")
                    nc.vector.tensor_tensor(out=s2[:], in0=s1[:],
                                            in1=za_sb[:, t * HID:(t + 1) * HID],
                                            op=mybir.AluOpType.add)
                    s3 = ev_p.tile([128, HID], f32, tag="s1")
                    nc.vector.tensor_tensor(out=s3[:], in0=s2[:], in1=AC1_sb[:, :HID],
                                            op=mybir.AluOpType.mult)
                    s4 = ev_p.tile([128, HID], f32, tag="s2")
                    nc.vector.tensor_tensor(out=s4[:], in0=s3[:], in1=AC1_sb[:, HID:],
                                            op=mybir.AluOpType.add)
                    h = ev_p.tile([128, HID], f16, tag="h")
                    nc.vector.tensor_scalar(h[:], s4[:], 0.0, None,
                                            mybir.AluOpType.max)
                    pst = ps_tr.tile([128, 128], f16)
                    nc.tensor.transpose(out=pst[:], in_=h[:], identity=ident[:])
                    nc.vector.tensor_copy(hT_sb[:, t * 128:(t + 1) * 128], pst[:])

                propagate(t2_t, HID, f16, g16_p, evac_p2)

                # ================= layer 2 =================
                za2_sb = big_p.tile([128, TILES * OUT], f32, tag="za")
                zb2_sb = big_p.tile([128, TILES * OUT], f32, tag="zb")

                def dense2():
                    for t in range(TILES):
                        lhsT = hT_sb[:, t * 128:(t + 1) * 128]
                        ps = ps_dense.tile([128, 3 * OUT], f32)
                        for j in range(3):
                            nc.tensor.matmul(ps[:, j * OUT:(j + 1) * OUT], lhsT,
                                             W2_sb[:, j * OUT:(j + 1) * OUT],
                                             start=True, stop=True)
                        nc.vector.tensor_copy(za2_sb[:, t * OUT:(t + 1) * OUT], ps[:, 0:OUT])
                        nc.vector.tensor_scalar(zb2_sb[:, t * OUT:(t + 1) * OUT],
                                                ps[:, OUT:2 * OUT], disP_sb[:, t:t + 1],
                                                None, mybir.AluOpType.mult)
                        zc = ev_p.tile([128, HID], f16, tag="zc")
                        nc.vector.tensor_scalar(zc[:, :OUT], ps[:, 2 * OUT:3 * OUT],
                                                disP_sb[:, t:t + 1], None,
                                                mybir.AluOpType.mult)
                        nc.vector.memset(zc[:, OUT:], 0.0)
                        nc.sync.dma_start(out=b3_t[t * 128:(t + 1) * 128, :],
                                          in_=zc[:, :])

                dense2()

                if single:
                    nc.sync.dma_start(out=t3_t[0:SHP, :], in_=b3_t[:, :])
                else:
                    nc.gpsimd.collective_compute(
                        "AllGather", mybir.AluOpType.bypass,
                        ins=[b3_t[:, :]], outs=[t3_t[:, :]], replica_groups=rg)

                def evac_p3(t, ps):
                    tmp = ev_p.tile([128, OUT], f32, tag="tmp32")
                    nc.vector.tensor_scalar(tmp[:], ps[:, :OUT], dis2N_sb[:, t:t + 1], None,
                                            mybir.AluOpType.mult)
                    v = ev_p.tile([128, HID], f16, tag="v16b")
                    nc.vector.tensor_tensor(out=v[:, :OUT], in0=tmp[:],
                                            in1=zb2_sb[:, t * OUT:(t + 1) * OUT],
                                            op=mybir.AluOpType.add)
                    nc.vector.memset(v[:, OUT:], 0.0)
                    nc.sync.dma_start(out=b4_t[t * 128:(t + 1) * 128, :], in_=v[:, :])

                propagate(t3_t, HID, f16, g16_p, evac_p3)

                if single:
                    nc.sync.dma_start(out=t4_t[0:SHP, :], in_=b4_t[:, :])
                else:
                    nc.gpsimd.collective_compute(
                        "AllGather", mybir.AluOpType.bypass,
                        ins=[b4_t[:, :]], outs=[t4_t[:, :]], replica_groups=rg)

                def evac_p4(t, ps):
                    o1 = ev_p.tile([128, OUT], f32, tag="o1")
                    nc.vector.tensor_scalar(o1[:], ps[:, :OUT], disN_sb[:, t:t + 1], None,
                                            mybir.AluOpType.mult)
                    o2 = ev_p.tile([128, OUT], f32, tag="o2")
                    nc.vector.tensor_tensor(out=o2[:], in0=o1[:],
                                            in1=za2_sb[:, t * OUT:(t + 1) * OUT],
                                            op=mybir.AluOpType.add)
                    o3 = ev_p.tile([128, OUT], f32, tag="o1")
                    nc.vector.tensor_tensor(out=o3[:], in0=o2[:], in1=AC2_sb[:, :OUT],
                                            op=mybir.AluOpType.mult)
                    o4 = ev_p.tile([128, OUT], f32, tag="o2")
                    nc.vector.tensor_tensor(out=o4[:], in0=o3[:], in1=AC2_sb[:, OUT:],
                                            op=mybir.AluOpType.add)
                    nc.sync.dma_start(out=out_d.ap()[t * 128:(t + 1) * 128, :], in_=o4[:])

                propagate(t4_t, HID, f16, g16_p, evac_p4)

            for _rep in range(repeat):
                run_body()


    if do_compile:
        nc.compile()
    return nc


# ---------------------------------------------------------------------------
# SPMD runner (axon / PJRT path), kept warm across calls
# ---------------------------------------------------------------------------


class SpmdRunner:
    def __init__(self, nc, n_cores=N_CORES):
        import jax
        from jax.sharding import Mesh, PartitionSpec, NamedSharding
        from jax.experimental.shard_map import shard_map
        from concourse.bass2jax import (_bass_exec_p, partition_id_tensor,
                                        install_neuronx_cc_hook)
        install_neuronx_cc_hook()
        self.jax = jax
        self.n_cores = n_cores
        partition_name = nc.partition_id_tensor.name if nc.partition_id_tensor else None
        in_names, out_names, out_avals, zero_outs = [], [], [], []
        for alloc in nc.m.functions[0].allocations:
            if not isinstance(alloc, mybir.MemoryLocationSet):
                continue
            name = alloc.memorylocations[0].name
            if alloc.kind == "ExternalInput":
                if name != partition_name:
                    in_names.append(name)
            elif alloc.kind == "ExternalOutput":
                out_names.append(name)
                shape = tuple(alloc.tensor_shape)
                dtype = mybir.dt.np(alloc.dtype)
                out_avals.append(jax.core.ShapedArray(shape, dtype))
                zero_outs.append(np.zeros(shape, dtype))
        self.in_names, self.out_names = in_names, out_names
        self.out_avals, self.zero_outs = out_avals, zero_outs
        all_in_names = list(in_names) + list(out_names)
        if partition_name is not None:
            all_in_names.append(partition_name)

        def _body(*args):
            operands = list(args)
            if partition_name is not None:
                operands.append(partition_id_tensor())
            outs = _bass_exec_p.bind(
                *operands,
                out_avals=tuple(out_avals),
                in_names=tuple(all_in_names),
                out_names=tuple(out_names),
                lowering_input_output_aliases=(),
                sim_require_finite=True,
                sim_require_nnan=True,
                nc=nc,
            )
            return tuple(outs)

        devices = jax.devices()[:n_cores]
        self.mesh = Mesh(np.asarray(devices), ("core",))
        spec = PartitionSpec("core")
        self.sharding = NamedSharding(self.mesh, spec)
        in_specs = (spec,) * (len(in_names) + len(out_names))
        out_specs = (spec,) * len(out_names)
        self.fn = jax.jit(
            shard_map(_body, mesh=self.mesh, in_specs=in_specs,
                      out_specs=out_specs, check_rep=False),
            keep_unused=True,
        )

    def stage(self, in_maps):
        concat_in = [
            np.concatenate([np.asarray(in_maps[c][n]) for c in range(self.n_cores)], axis=0)
            for n in self.in_names
        ]
        concat_zeros = [
            np.zeros((self.n_cores * z.shape[0], *z.shape[1:]), z.dtype)
            for z in self.zero_outs
        ]
        dev = [self.jax.device_put(a, self.sharding) for a in concat_in + concat_zeros]
        self.jax.block_until_ready(dev)
        return dev

    def run(self, staged):
        out = self.fn(*staged)
        self.jax.block_until_ready(out)
        return out

    def unpack(self, out_arrs):
        res = []
        for c in range(self.n_cores):
            d = {}
            for i, n in enumerate(self.out_names):
                d[n] = np.asarray(out_arrs[i]).reshape(
                    self.n_cores, *self.out_avals[i].shape)[c]
            res.append(d)
        return res


_CACHE = {}


def _get_runner(cfg, meta):
    key = (tuple(sorted(cfg.items())), meta["nb_shared"].tobytes())
    if key not in _CACHE:
        nc = build_program(cfg, meta)
        _CACHE[key] = SpmdRunner(nc)
    return _CACHE[key]


def run_model(x, edge_index, weights, cfg):
    meta, dis, idx_w, dloc_t = preprocess_edges(edge_index, cfg)
    in_maps = build_host_inputs(x, dis, weights, cfg)
    for c in range(N_CORES):
        in_maps[c]["idxs"] = idx_w[c]
        in_maps[c]["dloc"] = dloc_t[c]
    r = _get_runner(cfg, meta)
    staged = r.stage(in_maps)
    res = r.unpack(r.run(staged))
    N, SH, OUT = cfg["N"], cfg["SH"], cfg["OUT"]
    out = np.empty((N, OUT), np.float32)
    for c in range(N_CORES):
        out[c * SH:(c + 1) * SH] = res[c]["out"][:SH]
    return out


def kernel(x, edge_index, W1, b1, W2, b2, g1, beta1, m1, v1, g2, beta2, m2, v2):
    x = np.asarray(x, np.float32)
    edge_index = np.asarray(edge_index)
    weights = tuple(np.asarray(w, np.float32) for w in
                    (W1, b1, W2, b2, g1, beta1, m1, v1, g2, beta2, m2, v2))
    return run_model(x, edge_index, weights, CFG)



# revision 7
# speedup vs baseline: 11.4870x; 11.4870x over previous
"""ChebConv(K=3) x2 + BN GNN kernel for 8 Trainium2 NeuronCores.

Strategy:
  - Nodes dst-sharded across 8 cores (12500 each, padded to 12544 = 98*128).
  - ChebNet algebra refactored: out = x@(W0-W2) + L(x@W1) + L(L(x@(2W2)))
    with L = -D^-1/2 A D^-1/2 applied as: scale rows by dis on the way into
    the gather table, scale by -dis on the way out of the aggregation.
  - Each propagate: per dst-tile (128 dsts), gather source rows from a
    replicated table in HBM via dma_gather (int16 idx, 4 equal 25088-row
    regions), build a 0/1 selection matrix on DVE (iota==dstloc), and
    aggregate with TensorE matmuls accumulating in PSUM.
  - Gather calls are emitted REGION-ROTATED (consecutive calls hit different
    25088-row regions, queue == region). Measured 3.2x faster than
    same-region call runs (52 -> 166 GB/s/core).
  - Elementwise work during gather phases uses ACT activation ops (per-
    partition scale) and DVE tensor_tensor only; tensor_scalar/tensor_copy
    can enter DVE 2-port perf mode which fully blocks SWDGE desc-gen.
  - Layer-1 tables fp16 [TR,128] (256B rows); layer-2 tables fp32 [TR,64]
    (256B rows) -> identical row indexing, one shared idx/dloc stream.
"""
import sys

for _p in ("/opt/trn_rl_repo",):
    if _p not in sys.path:
        sys.path.insert(0, _p)

import numpy as np

import concourse.bass as bass
import concourse.bacc as bacc
import concourse.mybir as mybir
import concourse.tile as tile
from concourse.masks import make_identity

N_CORES = 8
EPS = 1e-5
NI_MAX_BATCHES = 8  # <=1024 idxs per dma_gather (HW ring limit)
SCRATCH = 16384  # SWDGE descriptor ring bytes/partition
GSZ = 4  # dst tiles per plan group


def make_cfg(N, DIN, HID, OUT):
    SH = N // N_CORES
    assert SH * N_CORES == N
    TILES = (SH + 127) // 128
    SHP = TILES * 128
    TR = N_CORES * SHP  # table rows
    RSZ = 2 * SHP  # gather region rows (= 2 shards, < 32768 for int16)
    NREG = TR // RSZ
    assert NREG * RSZ == TR and RSZ < 32768
    return dict(N=N, DIN=DIN, HID=HID, OUT=OUT, SH=SH, SHP=SHP, TILES=TILES,
                TR=TR, RSZ=RSZ, NREG=NREG)


CFG = make_cfg(100000, 128, 128, 64)

# ---------------------------------------------------------------------------
# Host preprocessing
# ---------------------------------------------------------------------------


def preprocess_edges(edge_index, cfg):
    """Group edges by (dst shard, dst tile, src region). Calls of up to
    NI_MAX_BATCHES*128 idxs cover per-(tile-group, region) runs; call
    emission rotates regions so concurrent gathers hit disjoint HBM spans."""
    N, SH, SHP, TILES = cfg["N"], cfg["SH"], cfg["SHP"], cfg["TILES"]
    RSZ, NREG = cfg["RSZ"], cfg["NREG"]
    src = edge_index[0].astype(np.int64)
    dst = edge_index[1].astype(np.int64)

    deg = np.bincount(src, minlength=N).astype(np.float64)
    dis = np.where(deg > 0, 1.0 / np.sqrt(np.maximum(deg, 1.0)), 0.0).astype(np.float32)

    shard = dst // SH
    tloc = (dst % SH) // 128
    rloc = (dst % SH) % 128
    rowof_all = (np.arange(N) // SH) * SHP + (np.arange(N) % SH)
    rowof = rowof_all[src]
    region = rowof // RSZ

    key = (shard * TILES + tloc) * NREG + region
    order = np.argsort(key, kind="stable")
    grp_cnt = np.bincount(key, minlength=N_CORES * TILES * NREG)
    grp_start = np.zeros(N_CORES * TILES * NREG + 1, np.int64)
    np.cumsum(grp_cnt, out=grp_start[1:])
    nb = -(-grp_cnt.reshape(N_CORES, TILES, NREG) // 128)  # ceil
    nb_shared = nb.max(axis=0)  # [TILES, NREG]
    nb_shared[:, 0] = np.maximum(nb_shared[:, 0], 1)  # every tile >=1 batch

    B_total = int(nb_shared.sum())
    S_total = B_total * 128

    # ---- stream layout + region-rotated gather plan ----
    run_off = np.zeros((TILES, NREG), np.int64)
    batch_tile = np.zeros(B_total, np.int32)
    plan = []  # list of (region, slot0, nb_i, gb0)
    last_gb = np.zeros(TILES, np.int64)
    first_gb = np.full(TILES, -1, np.int64)
    pos = 0  # slot position
    gb = 0
    for g0 in range(0, TILES, GSZ):
        g1 = min(g0 + GSZ, TILES)
        calls_by_region = []
        for r in range(NREG):
            run_nb = int(nb_shared[g0:g1, r].sum())
            p = pos
            bstart = gb
            for t in range(g0, g1):
                run_off[t, r] = p
                nbt = int(nb_shared[t, r])
                for _ in range(nbt):
                    batch_tile[gb] = t
                    if first_gb[t] < 0:
                        first_gb[t] = gb
                    last_gb[t] = gb
                    gb += 1
                p += nbt * 128
            calls = []
            sub = 0
            while sub < run_nb:
                nb_i = min(NI_MAX_BATCHES, run_nb - sub)
                calls.append((r, pos + sub * 128, nb_i, bstart + sub))
                sub += nb_i
            pos += run_nb * 128
            calls_by_region.append(calls)
        mx = max(len(c) for c in calls_by_region)
        for i in range(mx):
            for r in range(NREG):
                if i < len(calls_by_region[r]):
                    plan.append(calls_by_region[r][i])
    assert pos == S_total and gb == B_total

    idx_stream = np.zeros((N_CORES, S_total), np.int16)
    dloc_stream = np.full((N_CORES, S_total), 255, np.int16)
    src_local = (rowof % RSZ).astype(np.int16)
    s_sorted = src_local[order]
    r_sorted = rloc[order].astype(np.int16)

    for c in range(N_CORES):
        for t in range(TILES):
            for r in range(NREG):
                g = (c * TILES + t) * NREG + r
                n = grp_cnt[g]
                if n == 0:
                    continue
                a = grp_start[g]
                o = run_off[t, r]
                idx_stream[c, o:o + n] = s_sorted[a:a + n]
                dloc_stream[c, o:o + n] = r_sorted[a:a + n]

    idx_w = idx_stream.reshape(N_CORES, S_total // 16, 16).transpose(0, 2, 1)
    idx_w = np.ascontiguousarray(np.tile(idx_w, (1, 8, 1)))
    dloc_t = dloc_stream.reshape(N_CORES, B_total, 128).transpose(0, 2, 1)
    dloc_t = np.ascontiguousarray(dloc_t).astype(np.float16)

    meta = dict(nb_shared=nb_shared, B_total=B_total, S_total=S_total,
                plan=tuple(plan), batch_tile=tuple(batch_tile.tolist()),
                first_gb=tuple(first_gb.tolist()), last_gb=tuple(last_gb.tolist()))
    return meta, dis, idx_w, dloc_t


def build_host_inputs(x, dis, weights, cfg):
    """Per-core input tensors (excluding idx/dloc)."""
    (W1, b1, W2, b2, g1, beta1, m1, v1, g2, beta2, m2, v2) = weights
    N, SH, SHP, TILES = cfg["N"], cfg["SH"], cfg["SHP"], cfg["TILES"]
    DIN, HID, OUT = cfg["DIN"], cfg["HID"], cfg["OUT"]

    Wcat1 = np.concatenate([W1[0] - W1[2], W1[1], 2.0 * W1[2]], axis=1).astype(np.float16)
    Wcat2 = np.concatenate([W2[0] - W2[2], W2[1], 2.0 * W2[2]], axis=1).astype(np.float16)
    A1 = (g1 / np.sqrt(v1 + EPS)).astype(np.float32)
    C1 = (beta1 + (b1 - m1) * A1).astype(np.float32)
    A2 = (g2 / np.sqrt(v2 + EPS)).astype(np.float32)
    C2 = (beta2 + (b2 - m2) * A2).astype(np.float32)
    AC1 = np.concatenate([np.tile(A1, (128, 1)), np.tile(C1, (128, 1))], axis=1)
    AC2 = np.concatenate([np.tile(A2, (128, 1)), np.tile(C2, (128, 1))], axis=1)

    xp = np.zeros((N_CORES, SHP, DIN), np.float32)
    disp = np.zeros((N_CORES, SHP), np.float32)
    for c in range(N_CORES):
        xp[c, :SH] = x[c * SH:(c + 1) * SH]
        disp[c, :SH] = dis[c * SH:(c + 1) * SH]

    in_maps = []
    for c in range(N_CORES):
        d = disp[c].reshape(TILES, 128).T  # [128, TILES]
        in_maps.append({
            "xT": np.ascontiguousarray(xp[c].T).astype(np.float16),
            "disP": np.ascontiguousarray(d),
            "disN": np.ascontiguousarray(-d),
            "dis2N": np.ascontiguousarray(-(d.astype(np.float64) ** 2)).astype(np.float32),
            "Wcat1": Wcat1,
            "Wcat2": Wcat2,
            "AC1": AC1.astype(np.float32),
            "AC2": AC2.astype(np.float32),
        })
    return in_maps


# ---------------------------------------------------------------------------
# Bass program
# ---------------------------------------------------------------------------


def build_program(cfg, meta, repeat=1, parts="all", do_compile=True):
    dt = mybir.dt
    f16, f32, i16 = dt.float16, dt.float32, dt.int16
    SHP, TILES, TR = cfg["SHP"], cfg["TILES"], cfg["TR"]
    RSZ, NREG = cfg["RSZ"], cfg["NREG"]
    DIN, HID, OUT = cfg["DIN"], cfg["HID"], cfg["OUT"]
    B_total, S_total = meta["B_total"], meta["S_total"]
    AF = mybir.ActivationFunctionType

    nc = bacc.Bacc("TRN2", target_bir_lowering=False, debug=False,
                   num_devices=N_CORES, num_swdge_queues=4,
                   dynamic_dma_scratch_size=SCRATCH)

    xT_d = nc.dram_tensor("xT", [128, SHP], f16, kind="ExternalInput")
    disP_d = nc.dram_tensor("disP", [128, TILES], f32, kind="ExternalInput")
    disN_d = nc.dram_tensor("disN", [128, TILES], f32, kind="ExternalInput")
    dis2N_d = nc.dram_tensor("dis2N", [128, TILES], f32, kind="ExternalInput")
    W1_d = nc.dram_tensor("Wcat1", [DIN, 3 * HID], f16, kind="ExternalInput")
    W2_d = nc.dram_tensor("Wcat2", [HID, 3 * OUT], f16, kind="ExternalInput")
    AC1_d = nc.dram_tensor("AC1", [128, 2 * HID], f32, kind="ExternalInput")
    AC2_d = nc.dram_tensor("AC2", [128, 2 * OUT], f32, kind="ExternalInput")
    idx_d = nc.dram_tensor("idxs", [128, S_total // 16], i16, kind="ExternalInput")
    dloc_d = nc.dram_tensor("dloc", [128, B_total], f16, kind="ExternalInput")
    out_d = nc.dram_tensor("out", [SHP, OUT], f32, kind="ExternalOutput")

    rg = [list(range(N_CORES))]

    with tile.TileContext(nc) as tc:
        import contextlib
        ctx = contextlib.ExitStack()
        with ctx:
            const_p = ctx.enter_context(tc.tile_pool(name="const", bufs=1))
            big_p = ctx.enter_context(tc.tile_pool(name="big", bufs=1))
            g16_p = ctx.enter_context(tc.tile_pool(name="g16", bufs=12))
            s_p = ctx.enter_context(tc.tile_pool(name="sel", bufs=8))
            ev_p = ctx.enter_context(tc.tile_pool(name="ev", bufs=4))
            ps_prop = ctx.enter_context(tc.tile_pool(name="psprop", bufs=6, space="PSUM"))
            ps_dense = ctx.enter_context(tc.tile_pool(name="psdense", bufs=1, space="PSUM"))
            ps_tr = ctx.enter_context(tc.tile_pool(name="pstr", bufs=1, space="PSUM"))
            dram_p = ctx.enter_context(tc.tile_pool(name="dram", bufs=1, space="DRAM"))

            # ---- constants ----
            iota16_sb = const_p.tile([128, 128], f16)
            nc.gpsimd.iota(iota16_sb[:], pattern=[[1, 128]], base=0,
                           channel_multiplier=0,
                           allow_small_or_imprecise_dtypes=True)
            ident = const_p.tile([128, 128], f16)
            make_identity(nc, ident[:])
            W1_sb = const_p.tile([DIN, 3 * HID], f16)
            nc.sync.dma_start(out=W1_sb[:], in_=W1_d.ap())
            W2_sb = const_p.tile([HID, 3 * OUT], f16)
            nc.sync.dma_start(out=W2_sb[:], in_=W2_d.ap())
            AC1_sb = const_p.tile([128, 2 * HID], f32)
            nc.sync.dma_start(out=AC1_sb[:], in_=AC1_d.ap())
            AC2_sb = const_p.tile([128, 2 * OUT], f32)
            nc.sync.dma_start(out=AC2_sb[:], in_=AC2_d.ap())
            disP_sb = const_p.tile([128, TILES], f32)
            nc.sync.dma_start(out=disP_sb[:], in_=disP_d.ap())
            disN_sb = const_p.tile([128, TILES], f32)
            nc.sync.dma_start(out=disN_sb[:], in_=disN_d.ap())
            dis2N_sb = const_p.tile([128, TILES], f32)
            nc.sync.dma_start(out=dis2N_sb[:], in_=dis2N_d.ap())
            dloc16_sb = const_p.tile([128, B_total], f16)
            nc.sync.dma_start(out=dloc16_sb[:], in_=dloc_d.ap())
            idx_sb = const_p.tile([128, S_total // 16], i16)
            nc.sync.dma_start(out=idx_sb[:], in_=idx_d.ap())

            # ---- big resident arrays ----
            xT_sb = big_p.tile([128, SHP], f16, tag="xT")
            nc.sync.dma_start(out=xT_sb[:], in_=xT_d.ap())

            rep_i = [0]
            em_g = parts in ("all", "gather", "gs")
            em_s = parts in ("all", "gs")
            em_m = parts == "all"
            plan = meta["plan"]
            batch_tile = meta["batch_tile"]
            first_gb = meta["first_gb"]
            last_gb = meta["last_gb"]

            def run_body():
                ri = rep_i[0]
                rep_i[0] += 1
                zb_sb = big_p.tile([128, TILES * HID], f16, tag="zb")
                zac_sb = big_p.tile([128, TILES * HID], f16, tag="zac")
                b1_t = dram_p.tile([SHP, HID], f16, tag=f"b1_{ri}")
                t1_t = dram_p.tile([TR, HID], f16, addr_space="Shared", tag=f"t1_{ri}")
                b2_t = dram_p.tile([SHP, HID], f16, tag=f"b2_{ri}")
                t2_t = dram_p.tile([TR, HID], f16, addr_space="Shared", tag=f"t2_{ri}")
                b3_t = dram_p.tile([SHP, OUT], f32, tag=f"b3_{ri}")
                t3_t = dram_p.tile([TR, OUT], f32, addr_space="Shared", tag=f"t3_{ri}")
                b4_t = dram_p.tile([SHP, OUT], f32, tag=f"b4_{ri}")
                t4_t = dram_p.tile([TR, OUT], f32, addr_space="Shared", tag=f"t4_{ri}")

                def propagate(table, F, gdt, evac):
                    """y[dst] = sum_e table[src_e]; evac(t, psum) consumes PSUM."""
                    psums = {}
                    for (r, slot0, nb_i, gb0) in plan:
                        rows0 = r * RSZ
                        ni = nb_i * 128
                        col0 = slot0 // 16
                        g = g16_p.tile([128, NI_MAX_BATCHES, F], gdt, tag="g")
                        if em_g:
                            nc.gpsimd.dma_gather(
                                out_ap=g[:, :nb_i, :], in_ap=table[rows0:rows0 + RSZ, :],
                                idxs_ap=idx_sb[:, col0:col0 + ni // 16], num_idxs=ni,
                                num_idxs_reg=ni, elem_size=F,
                                queue_num=r)
                        if em_s:
                            Sw = s_p.tile([128, NI_MAX_BATCHES, 128], gdt, tag="S")
                            nc.vector.tensor_tensor(
                                out=Sw[:, :nb_i, :],
                                in0=iota16_sb[:].unsqueeze(1).broadcast_to([128, nb_i, 128]),
                                in1=dloc16_sb[:, gb0:gb0 + nb_i].unsqueeze(2).broadcast_to([128, nb_i, 128]),
                                op=mybir.AluOpType.is_equal)
                        if not em_m:
                            continue
                        for b in range(nb_i):
                            gb = gb0 + b
                            t = batch_tile[gb]
                            if t not in psums:
                                psums[t] = ps_prop.tile([128, F], f32, tag="pp", name=f"pp_{t}")
                            nc.tensor.matmul(psums[t][:], Sw[:, b, :], g[:, b, :],
                                             start=(gb == first_gb[t]),
                                             stop=(gb == last_gb[t]))
                            if gb == last_gb[t]:
                                evac(t, psums.pop(t))

                if parts == "ag":
                    # timing-only: 4 chained AllGathers of table-sized tensors
                    bb = dram_p.tile([SHP, HID], f16, tag=f"bb_{ri}")
                    nc.sync.dma_start(out=bb[0:128, :], in_=xT_sb[:, 0:HID])
                    prev = bb
                    for k in range(4):
                        tk = dram_p.tile([TR, HID], f16, addr_space="Shared",
                                         tag=f"tk{k}_{ri}")
                        nc.gpsimd.collective_compute(
                            "AllGather", mybir.AluOpType.bypass,
                            ins=[prev[0:SHP, :]], outs=[tk[:, :]],
                            replica_groups=rg)
                        prev = tk  # chain: next AG reads first SHP rows of last
                    return
                if not em_m:
                    # timing-only: 4 propagates' gather/S traffic vs a dummy table
                    dummy_t = dram_p.tile([TR, HID], f16, tag=f"dummy_{ri}")
                    nc.sync.dma_start(out=dummy_t[0:128, :],
                                      in_=xT_sb[:, 0:HID])
                    for _ in range(4):
                        propagate(dummy_t, HID, f16, None)
                    return

                # ================= layer 1 =================
                for t in range(TILES):
                    lhsT = xT_sb[:, t * 128:(t + 1) * 128]
                    ps = ps_dense.tile([128, 3 * HID], f32)
                    for j in range(3):
                        nc.tensor.matmul(ps[:, j * HID:(j + 1) * HID], lhsT,
                                         W1_sb[:, j * HID:(j + 1) * HID],
                                         start=True, stop=True)
                    nc.scalar.activation(zb_sb[:, t * HID:(t + 1) * HID],
                                         ps[:, HID:2 * HID], AF.Copy,
                                         scale=disP_sb[:, t:t + 1])
                    zc = ev_p.tile([128, HID], f16, tag="zc")
                    nc.scalar.activation(zc[:], ps[:, 2 * HID:3 * HID], AF.Copy,
                                         scale=disP_sb[:, t:t + 1])
                    nc.sync.dma_start(out=b1_t[t * 128:(t + 1) * 128, :], in_=zc[:])
                    s1 = ev_p.tile([128, HID], f32, tag="s1")
                    nc.vector.tensor_tensor(out=s1[:], in0=ps[:, 0:HID],
                                            in1=AC1_sb[:, :HID],
                                            op=mybir.AluOpType.mult)
                    nc.vector.tensor_tensor(out=zac_sb[:, t * HID:(t + 1) * HID],
                                            in0=s1[:], in1=AC1_sb[:, HID:],
                                            op=mybir.AluOpType.add)

                nc.gpsimd.collective_compute(
                    "AllGather", mybir.AluOpType.bypass,
                    ins=[b1_t[:, :]], outs=[t1_t[:, :]], replica_groups=rg)

                def evac_p1(t, ps):
                    tmp = ev_p.tile([128, HID], f16, tag="tmp16")
                    nc.scalar.activation(tmp[:], ps[:], AF.Copy,
                                         scale=dis2N_sb[:, t:t + 1])
                    v = ev_p.tile([128, HID], f16, tag="v16")
                    nc.vector.tensor_tensor(out=v[:], in0=tmp[:],
                                            in1=zb_sb[:, t * HID:(t + 1) * HID],
                                            op=mybir.AluOpType.add)
                    nc.sync.dma_start(out=b2_t[t * 128:(t + 1) * 128, :], in_=v[:])

                propagate(t1_t, HID, f16, evac_p1)

                nc.gpsimd.collective_compute(
                    "AllGather", mybir.AluOpType.bypass,
                    ins=[b2_t[:, :]], outs=[t2_t[:, :]], replica_groups=rg)

                hT_sb = big_p.tile([128, SHP], f16, tag=("xT" if repeat == 1 else "hT"))

                def evac_p2(t, ps):
                    s1 = ev_p.tile([128, HID], f32, tag="s1")
                    nc.scalar.activation(s1[:], ps[:], AF.Copy,
                                         scale=disN_sb[:, t:t + 1])
                    s2 = ev_p.tile([128, HID], f32, tag="s2")
                    nc.vector.tensor_tensor(out=s2[:], in0=s1[:], in1=AC1_sb[:, :HID],
                                            op=mybir.AluOpType.mult)
                    s3 = ev_p.tile([128, HID], f32, tag="s1")
                    nc.vector.tensor_tensor(out=s3[:], in0=s2[:],
                                            in1=zac_sb[:, t * HID:(t + 1) * HID],
                                            op=mybir.AluOpType.add)
                    h = ev_p.tile([128, HID], f16, tag="h")
                    nc.scalar.activation(h[:], s3[:], AF.Relu)
                    pst = ps_tr.tile([128, 128], f16)
                    nc.tensor.transpose(out=pst[:], in_=h[:], identity=ident[:])
                    nc.scalar.activation(hT_sb[:, t * 128:(t + 1) * 128], pst[:],
                                         AF.Copy)

                propagate(t2_t, HID, f16, evac_p2)

                # ================= layer 2 =================
                zb2_sb = big_p.tile([128, TILES * OUT], f32, tag="zb")
                zac2_sb = big_p.tile([128, TILES * OUT], f32, tag="zac")

                for t in range(TILES):
                    lhsT = hT_sb[:, t * 128:(t + 1) * 128]
                    ps = ps_dense.tile([128, 3 * HID], f32)
                    for j in range(3):
                        nc.tensor.matmul(ps[:, j * OUT:(j + 1) * OUT], lhsT,
                                         W2_sb[:, j * OUT:(j + 1) * OUT],
                                         start=True, stop=True)
                    nc.scalar.activation(zb2_sb[:, t * OUT:(t + 1) * OUT],
                                         ps[:, OUT:2 * OUT], AF.Copy,
                                         scale=disP_sb[:, t:t + 1])
                    zc = ev_p.tile([128, OUT], f32, tag="zc2")
                    nc.scalar.activation(zc[:], ps[:, 2 * OUT:3 * OUT], AF.Copy,
                                         scale=disP_sb[:, t:t + 1])
                    nc.sync.dma_start(out=b3_t[t * 128:(t + 1) * 128, :], in_=zc[:])
                    s1 = ev_p.tile([128, OUT], f32, tag="s1b")
                    nc.vector.tensor_tensor(out=s1[:], in0=ps[:, 0:OUT],
                                            in1=AC2_sb[:, :OUT],
                                            op=mybir.AluOpType.mult)
                    nc.vector.tensor_tensor(out=zac2_sb[:, t * OUT:(t + 1) * OUT],
                                            in0=s1[:], in1=AC2_sb[:, OUT:],
                                            op=mybir.AluOpType.add)

                nc.gpsimd.collective_compute(
                    "AllGather", mybir.AluOpType.bypass,
                    ins=[b3_t[:, :]], outs=[t3_t[:, :]], replica_groups=rg)

                def evac_p3(t, ps):
                    tmp = ev_p.tile([128, OUT], f32, tag="tmp32")
                    nc.scalar.activation(tmp[:], ps[:], AF.Copy,
                                         scale=dis2N_sb[:, t:t + 1])
                    v = ev_p.tile([128, OUT], f32, tag="v32")
                    nc.vector.tensor_tensor(out=v[:], in0=tmp[:],
                                            in1=zb2_sb[:, t * OUT:(t + 1) * OUT],
                                            op=mybir.AluOpType.add)
                    nc.sync.dma_start(out=b4_t[t * 128:(t + 1) * 128, :], in_=v[:])

                propagate(t3_t, OUT, f32, evac_p3)

                nc.gpsimd.collective_compute(
                    "AllGather", mybir.AluOpType.bypass,
                    ins=[b4_t[:, :]], outs=[t4_t[:, :]], replica_groups=rg)

                def evac_p4(t, ps):
                    o1 = ev_p.tile([128, OUT], f32, tag="o1")
                    nc.scalar.activation(o1[:], ps[:], AF.Copy,
                                         scale=disN_sb[:, t:t + 1])
                    o2 = ev_p.tile([128, OUT], f32, tag="o2")
                    nc.vector.tensor_tensor(out=o2[:], in0=o1[:], in1=AC2_sb[:, :OUT],
                                            op=mybir.AluOpType.mult)
                    o3 = ev_p.tile([128, OUT], f32, tag="o1")
                    nc.vector.tensor_tensor(out=o3[:], in0=o2[:],
                                            in1=zac2_sb[:, t * OUT:(t + 1) * OUT],
                                            op=mybir.AluOpType.add)
                    nc.sync.dma_start(out=out_d.ap()[t * 128:(t + 1) * 128, :], in_=o3[:])

                propagate(t4_t, OUT, f32, evac_p4)

            for _rep in range(repeat):
                run_body()

    if do_compile:
        nc.compile()
    return nc


# ---------------------------------------------------------------------------
# SPMD runner (axon / PJRT path), kept warm across calls
# ---------------------------------------------------------------------------


class SpmdRunner:
    def __init__(self, nc, n_cores=N_CORES):
        import jax
        from jax.sharding import Mesh, PartitionSpec, NamedSharding
        from jax.experimental.shard_map import shard_map
        from concourse.bass2jax import (_bass_exec_p, partition_id_tensor,
                                        install_neuronx_cc_hook)
        install_neuronx_cc_hook()
        self.jax = jax
        self.n_cores = n_cores
        partition_name = nc.partition_id_tensor.name if nc.partition_id_tensor else None
        in_names, out_names, out_avals, zero_outs = [], [], [], []
        for alloc in nc.m.functions[0].allocations:
            if not isinstance(alloc, mybir.MemoryLocationSet):
                continue
            name = alloc.memorylocations[0].name
            if alloc.kind == "ExternalInput":
                if name != partition_name:
                    in_names.append(name)
            elif alloc.kind == "ExternalOutput":
                out_names.append(name)
                shape = tuple(alloc.tensor_shape)
                dtype = mybir.dt.np(alloc.dtype)
                out_avals.append(jax.core.ShapedArray(shape, dtype))
                zero_outs.append(np.zeros(shape, dtype))
        self.in_names, self.out_names = in_names, out_names
        self.out_avals, self.zero_outs = out_avals, zero_outs
        all_in_names = list(in_names) + list(out_names)
        if partition_name is not None:
            all_in_names.append(partition_name)

        def _body(*args):
            operands = list(args)
            if partition_name is not None:
                operands.append(partition_id_tensor())
            outs = _bass_exec_p.bind(
                *operands,
                out_avals=tuple(out_avals),
                in_names=tuple(all_in_names),
                out_names=tuple(out_names),
                lowering_input_output_aliases=(),
                sim_require_finite=True,
                sim_require_nnan=True,
                nc=nc,
            )
            return tuple(outs)

        devices = jax.devices()[:n_cores]
        self.mesh = Mesh(np.asarray(devices), ("core",))
        spec = PartitionSpec("core")
        self.sharding = NamedSharding(self.mesh, spec)
        in_specs = (spec,) * (len(in_names) + len(out_names))
        out_specs = (spec,) * len(out_names)
        self.fn = jax.jit(
            shard_map(_body, mesh=self.mesh, in_specs=in_specs,
                      out_specs=out_specs, check_rep=False),
            keep_unused=True,
        )

    def stage(self, in_maps):
        concat_in = [
            np.concatenate([np.asarray(in_maps[c][n]) for c in range(self.n_cores)], axis=0)
            for n in self.in_names
        ]
        concat_zeros = [
            np.zeros((self.n_cores * z.shape[0], *z.shape[1:]), z.dtype)
            for z in self.zero_outs
        ]
        dev = [self.jax.device_put(a, self.sharding) for a in concat_in + concat_zeros]
        self.jax.block_until_ready(dev)
        return dev

    def run(self, staged):
        out = self.fn(*staged)
        self.jax.block_until_ready(out)
        return out

    def unpack(self, out_arrs):
        res = []
        for c in range(self.n_cores):
            d = {}
            for i, n in enumerate(self.out_names):
                d[n] = np.asarray(out_arrs[i]).reshape(
                    self.n_cores, *self.out_avals[i].shape)[c]
            res.append(d)
        return res


_CACHE = {}


def _get_runner(cfg, meta):
    key = (tuple(sorted(cfg.items())), meta["nb_shared"].tobytes())
    if key not in _CACHE:
        nc = build_program(cfg, meta)
        _CACHE[key] = SpmdRunner(nc)
    return _CACHE[key]


def run_model(x, edge_index, weights, cfg):
    meta, dis, idx_w, dloc_t = preprocess_edges(edge_index, cfg)
    in_maps = build_host_inputs(x, dis, weights, cfg)
    for c in range(N_CORES):
        in_maps[c]["idxs"] = idx_w[c]
        in_maps[c]["dloc"] = dloc_t[c]
    r = _get_runner(cfg, meta)
    staged = r.stage(in_maps)
    res = r.unpack(r.run(staged))
    N, SH, OUT = cfg["N"], cfg["SH"], cfg["OUT"]
    out = np.empty((N, OUT), np.float32)
    for c in range(N_CORES):
        out[c * SH:(c + 1) * SH] = res[c]["out"][:SH]
    return out


def kernel(x, edge_index, W1, b1, W2, b2, g1, beta1, m1, v1, g2, beta2, m2, v2):
    x = np.asarray(x, np.float32)
    edge_index = np.asarray(edge_index)
    weights = tuple(np.asarray(w, np.float32) for w in
                    (W1, b1, W2, b2, g1, beta1, m1, v1, g2, beta2, m2, v2))
    return run_model(x, edge_index, weights, CFG)
